# revision 1
# baseline (speedup 1.0000x reference)
"""Trainium2 Bass kernel for nn_Attention_712964571585.

Grouped multi-head attention with RoPE and null-KV, B=4 G=2 N=2048 D=512
H=8 DH=64. Sharded data-parallel over B*G = 8 NeuronCores (core c handles
b=c//2, g=c%2); each core runs the full per-(b,g) attention block.

Schedule: the Act engine (exp of all S^T logits, 272 x [128,1024]
instructions ~= 284us) and the PE (projections + S + PV + out-proj
~= 285us) are balanced rails; everything is software-pipelined so both
stay busy:

  - QKV projections as bf16 matmuls; rotate-half is ONE extra [128,128]
    permutation matmul per 512-chunk (prot matrix, signs included)
    instead of a full second projection (saves ~24us PE vs rotated
    weight copies); rope combine m*cos + rot*sin on DVE (bf16 4x modes
    where possible).
  - Attention in S^T layout ([k, q], k on partitions): softmax without
    max-subtraction; exp on Act (PSUM->SBUF bf16, one [128, 2*QC]
    instruction per k-block covering both heads of a pair); denominator
    via ones-column in V' (PV matmul M=65); reciprocal + DRAM-roundtrip
    broadcast, normalize mults in bf16.
  - The null key/value is a 17th k-block (row 0 = null kv, rows 1..127
    masked via per-partition exp bias of -60).
  - Pipelining: q/k projection of pair (p+1)%4 and V'-projection of the
    next loop iteration are emitted as filler inside attention k-loops;
    out-projection of pair p-1 fills the exp-wait bubble at each
    q-chunk start (PSUM tile rings shared with the attention o tiles:
    exactly 8 PSUM banks). Normalize mults are deferred one q-chunk so
    the DRAM-broadcast latency stays off the critical path.
"""
import numpy as np
import ml_dtypes
import concourse.bass as bass
import concourse.mybir as mybir
from concourse import bacc
from concourse.tile import TileContext
from concourse.bass_utils import run_bass_kernel_spmd

F32 = mybir.dt.float32
BF16 = mybir.dt.bfloat16
AF = mybir.ActivationFunctionType
MULT = mybir.AluOpType.mult
ADD = mybir.AluOpType.add

B, G, N, D = 4, 2, 2048, 512
H, DH = 8, 64
DI = H * DH
DB = D // 128          # 4 d-blocks
EB = DI // 128         # 4 e-blocks (= head pairs)
NB = N // 128          # 16 n-blocks
NKB = NB + 1           # 17 k-blocks (incl null)
VW = DH + 1            # 65: V plus ones column
NEG = -60.0

# debug hook: simlbl.py overrides this to tag instructions with context
_label = lambda s: None


def build_nc(QC=512, n_time_loops=1, unroll=False, variant=""):
    """Build the per-core Bass graph. QC: q-chunk size (one exp instruction
    covers both heads' S^T of one k-block: [128, 2*QC]). unroll=True
    repeats the body as straight-line code (for local timeline-sim)."""
    NQC = N // QC                   # q-chunks
    nc = bacc.Bacc(None)

    xT_d = nc.dram_tensor("xT", [D, N], BF16, kind="ExternalInput")
    wq_d = nc.dram_tensor("wq", [D, DI], BF16, kind="ExternalInput")
    wk_d = nc.dram_tensor("wk", [D, DI], BF16, kind="ExternalInput")
    wv_d = nc.dram_tensor("wv", [D, DI], BF16, kind="ExternalInput")
    wout_d = nc.dram_tensor("wout", [DI, D], BF16, kind="ExternalInput")
    prot_d = nc.dram_tensor("prot", [128, 128], BF16, kind="ExternalInput")
    cosq_d = nc.dram_tensor("cosq", [128, N], BF16, kind="ExternalInput")
    sinq_d = nc.dram_tensor("sinq", [128, N], BF16, kind="ExternalInput")
    cosk_d = nc.dram_tensor("cosk", [128, N], BF16, kind="ExternalInput")
    sink_d = nc.dram_tensor("sink", [128, N], BF16, kind="ExternalInput")
    nkp_d = nc.dram_tensor("nkp", [128, EB], F32, kind="ExternalInput")
    nvf_d = nc.dram_tensor("nvf", [1, DI], F32, kind="ExternalInput")
    nbias_d = nc.dram_tensor("nbias", [128, 1], F32, kind="ExternalInput")
    yT_d = nc.dram_tensor("out", [D, N], F32, kind="ExternalOutput")

    with TileContext(nc) as tc:
        with (
            tc.tile_pool(name="persist", bufs=1) as pp,
            tc.tile_pool(name="etile", bufs=(8 if "ep8" in variant else 5)) as ep,
            tc.tile_pool(name="small", bufs=2) as mp,
            tc.tile_pool(name="actp", bufs=2) as ap_pool,
            tc.tile_pool(name="psA", bufs=2, space="PSUM") as psA,   # stAB [128,1024] x2 = 4 banks
            tc.tile_pool(name="psM", bufs=1, space="PSUM") as psM,   # m_ps [128,512] = 1 bank
            tc.tile_pool(name="psR", bufs=1, space="PSUM") as psR,   # r_ps [128,512] = 1 bank
            tc.tile_pool(name="psC", bufs=1, space="PSUM") as psC,   # o_A/o_B + outproj = 2 banks
            tc.tile_pool(name="drp", bufs=2, space="DRAM") as drp,
        ):
            # ---- persistent tiles (bf16 direct loads; x first for startup) ----
            xTb = [pp.tile([128, N], BF16, tag=f"xT{i}", name=f"xT{i}") for i in range(DB)]
            for i in range(DB):
                nc.sync.dma_start(xTb[i][:], xT_d[i * 128:(i + 1) * 128, :])
            wtiles = {}
            for wname, dram in (("wv", wv_d), ("wk", wk_d),
                                ("wq", wq_d), ("wout", wout_d)):
                bf = [pp.tile([128, 512], BF16, tag=f"{wname}b{i}", name=f"{wname}b{i}")
                      for i in range(4)]
                wtiles[wname] = bf
                for i in range(4):
                    nc.sync.dma_start(bf[i][:], dram[i * 128:(i + 1) * 128, :])
            prot = pp.tile([128, 128], BF16, tag="prot", name="prot")
            nc.sync.dma_start(prot[:], prot_d[:])
            cosq = pp.tile([128, N], BF16, tag="cosq", name="cosq")
            sinq = pp.tile([128, N], BF16, tag="sinq", name="sinq")
            cosk = pp.tile([128, N], BF16, tag="cosk", name="cosk")
            sink = pp.tile([128, N], BF16, tag="sink", name="sink")
            nc.sync.dma_start(cosk[:], cosk_d[:])
            nc.sync.dma_start(sink[:], sink_d[:])
            nc.sync.dma_start(cosq[:], cosq_d[:])
            nc.sync.dma_start(sinq[:], sinq_d[:])
            nbias = pp.tile([128, 1], F32, tag="nbias", name="nbias")
            nc.sync.dma_start(nbias[:], nbias_d[:])
            nkp = pp.tile([128, EB], F32, tag="nkp", name="nkp")
            nc.sync.dma_start(nkp[:], nkp_d[:])
            nvf = pp.tile([1, DI], F32, tag="nvf", name="nvf")
            nc.sync.dma_start(nvf[:], nvf_d[:])

            y_acc = [pp.tile([128, N], F32, tag=f"yac{i}", name=f"yac{i}")
                     for i in range(DB)]
            # V': [128, H*VW] per n-block (17th = null)
            Vp = [pp.tile([128, H * VW], BF16, tag=f"Vp{nb}", name=f"Vp{nb}")
                  for nb in range(NKB)]

            qT, kT, qTB, kTB, OT = {}, {}, {}, {}, {}

            # ---------------- emission helpers ----------------
            def emit_vproj(nb):
                _label(f'vproj{nb}')
                if nb == NB:    # null block
                    vt = Vp[NB][:].rearrange("p (h w) -> p h w", h=H)
                    nc.vector.memset(Vp[NB][:], 0.0)
                    nc.vector.memset(vt[:, :, DH:VW], 1.0)
                    nc.vector.tensor_copy(
                        vt[0:1, :, 0:DH],
                        nvf[:].rearrange("p (h j) -> p h j", h=H))
                    return
                v_ps = psM.tile([128, 512], F32, tag="mps", name="mps")
                for db in range(DB):
                    nc.tensor.matmul(
                        v_ps, xTb[db][:, nb * 128:(nb + 1) * 128],
                        wtiles["wv"][db][:],
                        start=(db == 0), stop=(db == DB - 1))
                vt = Vp[nb][:].rearrange("p (h w) -> p h w", h=H)
                nc.vector.memset(vt[:, :, DH:VW], 1.0)
                nc.vector.tensor_copy(
                    vt[:, :, 0:DH],
                    v_ps.rearrange("p (h j) -> p h j", h=H))

            def make_proj_items(p):
                """9 filler closures projecting q/k for pair p (chunks
                j=0..3: k, 4..7: q; rot matmul staggered one chunk)."""
                qT[p] = ap_pool.tile([128, N], BF16, tag="qT", name="qT")
                kT[p] = ap_pool.tile([128, N + 128], BF16, tag="kT", name="kT")
                qTB[p] = ap_pool.tile([64, N], BF16, tag="qTB", name="qTB")
                kTB[p] = ap_pool.tile([64, N + 128], BF16, tag="kTB", name="kTB")
                OT[p] = ap_pool.tile([128, N], BF16, tag="OT", name="OT")
                e0 = p * 128
                pend = {}

                def stageA(j):
                    _label(f'projA p{p} j{j}')
                    ncki = j % 4
                    s = slice(ncki * 512, (ncki + 1) * 512)
                    wmain = wtiles["wk" if j < 4 else "wq"]
                    m_ps = psM.tile([128, 512], F32, tag="mps", name="mps")
                    for db in range(DB):
                        nc.tensor.matmul(
                            m_ps, wmain[db][:, e0:e0 + 128], xTb[db][:, s],
                            start=(db == 0), stop=(db == DB - 1))
                    qm = mp.tile([128, 512], BF16, tag="qmain", name="qmain")
                    nc.vector.tensor_copy(qm[:], m_ps)
                    pend[j] = qm

                def stageCD(j):
                    _label(f'projCD p{p} j{j}')
                    ncki = j % 4
                    s = slice(ncki * 512, (ncki + 1) * 512)
                    qm = pend.pop(j)
                    r_ps = psR.tile([128, 512], F32, tag="rps", name="rps")
                    nc.tensor.matmul(r_ps, prot[:], qm[:], start=True, stop=True)
                    ctab, stab = (cosk, sink) if j < 4 else (cosq, sinq)
                    dstT = kT[p] if j < 4 else qT[p]
                    m1 = mp.tile([128, 512], BF16, tag="m1", name="m1")
                    m2 = mp.tile([128, 512], BF16, tag="m2", name="m2")
                    nc.vector.tensor_tensor(m1[:], qm[:], ctab[:, s], MULT)
                    nc.vector.tensor_tensor(m2[:], r_ps, stab[:, s], MULT)
                    nc.vector.tensor_tensor(dstT[:, s], m1[:], m2[:], ADD)

                def make_item(j):
                    def f():
                        if j > 0:
                            stageCD(j - 1)
                        if j < 8:
                            stageA(j)
                        else:
                            nc.vector.memset(kT[p][:, N:N + 128], 0.0)
                            nc.vector.tensor_copy(kT[p][:, N:N + 1],
                                                  nkp[:, p:p + 1])
                            nc.sync.dma_start(qTB[p][:], qT[p][64:128, :])
                            nc.sync.dma_start(kTB[p][:], kT[p][64:128, :])
                    return f
                return [make_item(j) for j in range(9)]

            def emit_outproj_mm(pm1, ncki, dbk, half):
                _label(f'opj_mm p{pm1} n{ncki} d{dbk}')
                s = slice(ncki * 512, (ncki + 1) * 512)
                nc.tensor.matmul(
                    half, wtiles["wout"][pm1][:, dbk * 128:(dbk + 1) * 128],
                    OT[pm1][:, s], start=True, stop=True)
                return s

            def emit_outproj_dve(pm1, dbk, half, s):
                _label(f'opj_dve p{pm1} d{dbk}')
                if pm1 == 0:
                    nc.vector.tensor_copy(y_acc[dbk][:, s], half[:])
                else:
                    nc.vector.tensor_tensor(
                        y_acc[dbk][:, s], y_acc[dbk][:, s], half[:], ADD)
                if pm1 == EB - 1:
                    nc.sync.dma_start(
                        yT_d[dbk * 128:(dbk + 1) * 128, s], y_acc[dbk][:, s])

            def norm_copies(p, o_A, o_B):
                _label(f'normc p{p}')
                ocp = mp.tile([65, 2 * QC], BF16, tag="ocp", name="ocp")
                nc.vector.tensor_copy(ocp[:, 0:QC], o_A[:])
                nc.vector.tensor_copy(ocp[:, QC:2 * QC], o_B[:])
                return ocp

            def norm_recip(p, ocp):
                _label(f'normr p{p}')
                rch = mp.tile([128, 2 * QC], BF16, tag="reciph", name="reciph")
                with nc.allow_low_precision(reason="softmax denom in bf16"):
                    nc.vector.reciprocal(rch[64:65, :], ocp[64:65, :])
                scr = drp.tile([1, 2 * QC], BF16, tag="scr", name="scr")
                nc.sync.dma_start(scr[:], rch[64:65, :])
                rb = mp.tile([64, 2 * QC], BF16, tag="rbcast", name="rbcast")
                nc.sync.dma_start(rb[:], scr[:].to_broadcast([64, 2 * QC]))
                return rb

            def norm_tail(p, qc, ocp, rb):
                _label(f'normt p{p} q{qc}')
                qs = slice(qc * QC, (qc + 1) * QC)
                nc.vector.tensor_tensor(
                    OT[p][0:64, qs], ocp[0:64, 0:QC], rb[:, 0:QC], MULT)
                # head B written straight to partitions 64:128 (shifted
                # output base is legal; only dual-SBUF *inputs* must match)
                nc.vector.tensor_tensor(
                    OT[p][64:128, qs], ocp[0:64, QC:2 * QC],
                    rb[:, QC:2 * QC], MULT)

            eabs_g = {}      # (p, qc, kb) -> eAB tile (incl. lookahead)

            def emit_S(p, qc, kb):
                _label(f'S p{p} q{qc} k{kb}')
                qs = slice(qc * QC, (qc + 1) * QC)
                ks = slice(kb * 128, (kb + 1) * 128)
                stAB = psA.tile([128, 2 * QC], F32, tag="stAB", name="stAB")
                nc.tensor.matmul(
                    stAB[:, 0:QC], kT[p][0:64, ks],
                    qT[p][0:64, qs], start=True, stop=True)
                nc.tensor.matmul(
                    stAB[:, QC:2 * QC], kTB[p][:, ks],
                    qTB[p][:, qs], start=True, stop=True)
                bias = nbias[:, 0:1] if kb == NKB - 1 else 0.0
                eAB = ep.tile([128, 2 * QC], BF16, tag="eAB", name="eAB")
                if "noexp" in variant:
                    nc.scalar.activation(eAB[:, 0:8], stAB[:, 0:8],
                                         AF.Exp, bias=bias)
                else:
                    nc.scalar.activation(eAB[:], stAB[:], AF.Exp, bias=bias)
                eabs_g[(p, qc, kb)] = eAB

            def attention_pair(p, slots, qc_order, carry, lookahead=None):
                """Attention for pair p. slots: {(pos, kb)->[closures]} filler
                emissions. lookahead: next (p', qc') whose S(0)/S(1) should be
                emitted during this pair's last q-chunk (cross-pair software
                pipelining); within the pair, the next qc's S(0)/S(1) are
                always emitted at kb 15/16 of the previous qc."""
                pm1 = p - 1            # out-projection source (none for p=0)
                pending_recip, pending_mults = carry

                for pos in range(NQC):
                    qc = qc_order[pos]
                    o_A = psC.tile([65, QC], F32, tag="opsA", name="opsA")
                    o_B = psC.tile([65, QC], F32, tag="opsB", name="opsB")

                    if (p, qc, 0) not in eabs_g:
                        emit_S(p, qc, 0)
                        emit_S(p, qc, 1)
                    # qc-start fillers (out-projections etc.)
                    for fn in slots.get((pos, -1), ()):
                        fn()
                    if pending_recip[0] is not None:
                        pending_mults[0] = pending_recip[0]()
                        pending_recip[0] = None

                    for kb in range(NKB):
                        _label(f'PV p{p} q{qc} k{kb}')
                        eAB = eabs_g.pop((p, qc, kb))
                        vA = Vp[kb][:, (2 * p) * VW:(2 * p + 1) * VW]
                        vB = Vp[kb][:, (2 * p + 1) * VW:(2 * p + 2) * VW]
                        nc.tensor.matmul(
                            o_A, vA, eAB[:, 0:QC],
                            start=(kb == 0), stop=(kb == NKB - 1))
                        nc.tensor.matmul(
                            o_B, vB, eAB[:, QC:2 * QC],
                            start=(kb == 0), stop=(kb == NKB - 1))
                        if kb == 3 and pending_mults[0] is not None:
                            pending_mults[0]()
                            pending_mults[0] = None
                        for fn in slots.get((pos, kb), ()):
                            fn()
                        if kb + 2 < NKB:
                            emit_S(p, qc, kb + 2)
                        else:
                            # lookahead: S(0)/S(1) of the next q-chunk (or of
                            # the next pair) keep the Act pipeline full
                            nkb = kb + 2 - NKB
                            if pos + 1 < NQC:
                                emit_S(p, qc_order[pos + 1], nkb)
                            elif lookahead is not None:
                                emit_S(lookahead[0], lookahead[1], nkb)

                    # normalize: psum->sbuf copies now; recip+broadcast
                    # and the mults are deferred into the next qc
                    ocp = norm_copies(p, o_A, o_B)

                    def _recip(q_=qc, o_=ocp):
                        rb = norm_recip(p, o_)
                        return lambda: norm_tail(p, q_, o_, rb)
                    pending_recip[0] = _recip

            def emit_opj(pm1, ncki, dbks):
                """One out-projection sub-group: matmuls + y-acc DVE ops."""
                halves = []
                for dbk in dbks:
                    pool = psM if dbk % 2 == 0 else psR
                    tg = "mps" if dbk % 2 == 0 else "rps"
                    halves.append(pool.tile([128, 512], F32, tag=tg, name="opsH"))
                ss = [emit_outproj_mm(pm1, ncki, dbk, h)
                      for dbk, h in zip(dbks, halves)]
                for dbk, h, s in zip(dbks, halves, ss):
                    emit_outproj_dve(pm1, dbk, h, s)

            def pair_schedule(p, proj_items):
                """qc_order + filler slot map for attention_pair(p)."""
                qc_order = (0, 1, 2, 3)
                slots = {}

                def add(pos, kb, fn):
                    slots.setdefault((pos, kb), []).append(fn)

                def opj(pm1_, ncki, dbks):
                    return lambda: emit_opj(pm1_, ncki, dbks)

                if p == 0:
                    # V' blocks 6..16 just-in-time for the first q-chunk
                    for kb in range(11):
                        add(0, kb, lambda nb=kb + 6: emit_vproj(nb))
                    for i, it in enumerate(proj_items):
                        pos, kb = ((1, (2, 6, 10, 14)[i]) if i < 4 else
                                   (2, (2, 6, 10, 14)[i - 4]) if i < 8 else
                                   (3, 2))
                        add(pos, kb, it)
                elif p in (1, 2):
                    # out-proj of p-1, one column group per qc start
                    for pos in range(4):
                        add(pos, -1, opj(p - 1, pos, range(DB)))
                    for i, it in enumerate(proj_items):
                        add(i // 3, (2, 6, 10)[i % 3], it)
                else:
                    for pos in range(4):
                        add(pos, -1, opj(p - 1, pos, range(DB)))
                    for i, it in enumerate(proj_items):
                        add(i // 3, (2, 6, 10)[i % 3], it)
                    # own out-projection columns hoist off the epilogue as
                    # soon as their OT columns flush (at kb3 of the next qc)
                    for pos in (1, 2, 3):
                        add(pos, 6, opj(3, pos - 1, (0, 1)))
                        add(pos, 9, opj(3, pos - 1, (2, 3)))
                    # V' blocks 0..5 for the next loop iteration (after the
                    # last PV reads of this iteration, i.e. final pos)
                    for j, kb in enumerate((2, 2, 6, 6, 10, 10)):
                        add(3, kb, lambda nb=j: emit_vproj(nb))
                return qc_order, slots

            # ---------------- one-time prologue (iteration 0 inputs) -------
            prologue_items = make_proj_items(0)
            for it in prologue_items:
                it()
            for nb in range(NKB if "nofill" in variant else 6):
                emit_vproj(nb)

            import contextlib
            loop_ctx = (tc.For_i(0, n_time_loops, 1)
                        if n_time_loops > 1 and not unroll
                        else contextlib.nullcontext())
            n_unroll = n_time_loops if unroll else 1
            with loop_ctx:
              for _rep in range(n_unroll):
                  carry = [[None], [None]]
                  if "nofill" in variant:
                      for p_ in range(1, EB):
                          qT[p_] = qT[0]; kT[p_] = kT[0]
                          qTB[p_] = qTB[0]; kTB[p_] = kTB[0]
                          OT[p_] = OT[0]
                  for p in range(EB):
                      if "nofill" in variant:
                          # timing ablation: no proj/vproj/outproj fillers
                          la = (p + 1, 0) if p < EB - 1 else None
                          attention_pair(p, {}, (0, 1, 2, 3), carry, la)
                          continue
                      # projection filler for the next pair; at p==3 this
                      # projects pair 0 of the NEXT loop iteration
                      proj_items = make_proj_items((p + 1) % EB)
                      qc_order, slots = pair_schedule(p, proj_items)
                      la = (p + 1, 0) if p < EB - 1 else None
                      attention_pair(p, slots, qc_order, carry, la)
                  # epilogue: flush deferred normalize of (p3, last qc=2),
                  # then the final out-projection column group + y DMA
                  if carry[0][0] is not None:
                      carry[1][0] = carry[0][0]()
                      carry[0][0] = None
                  if carry[1][0] is not None:
                      carry[1][0]()
                      carry[1][0] = None
                  if "nofill" not in variant:
                      emit_opj(3, 3, (0, 1))
                      emit_opj(3, 3, (2, 3))

    return nc


# ---------------- host-side prep ----------------

def host_prep_shared(Wq, Wkv, Wout, null_kv, rot_q, rot_k):
    """Per-group (g) tensors shared by all cores of that group."""
    bf = ml_dtypes.bfloat16
    scale = DH ** -0.5
    # rotate-half permutation (with signs) as a [128,128] matmul operand:
    # r = P^T q with r[m] = -q[m+32] (m%64<32), +q[m-32] (m%64>=32)
    P = np.zeros((128, 128), np.float32)
    for b in (0, 64):
        for j in range(32):
            P[b + j + 32, b + j] = -1.0
            P[b + j, b + j + 32] = 1.0
    shared = []
    for g in range(G):
        d = {}
        Wqg = np.asarray(Wq[g], np.float32)
        Wk_, Wv_ = np.asarray(Wkv[g][:DI], np.float32), np.asarray(Wkv[g][DI:], np.float32)
        d["wq"] = np.ascontiguousarray(Wqg.T).astype(bf)
        d["wk"] = np.ascontiguousarray(Wk_.T).astype(bf)
        d["wv"] = np.ascontiguousarray(Wv_.T).astype(bf)
        d["wout"] = np.ascontiguousarray(np.asarray(Wout[g], np.float32).T).astype(bf)
        d["prot"] = P.astype(bf)
        cq = np.cos(rot_q).T.astype(np.float32) * scale
        sq = np.sin(rot_q).T.astype(np.float32) * scale
        ck = np.cos(rot_k).T.astype(np.float32)
        sk = np.sin(rot_k).T.astype(np.float32)
        d["cosq"] = np.ascontiguousarray(np.concatenate([cq, cq], 0)).astype(bf)
        d["sinq"] = np.ascontiguousarray(np.concatenate([sq, sq], 0)).astype(bf)
        d["cosk"] = np.ascontiguousarray(np.concatenate([ck, ck], 0)).astype(bf)
        d["sink"] = np.ascontiguousarray(np.concatenate([sk, sk], 0)).astype(bf)
        nk = np.asarray(null_kv[0, g, :, 0, :], np.float32)      # [H, DH]
        nv = np.asarray(null_kv[1, g, :, 0, :], np.float32)
        nkp = np.empty((128, EB), np.float32)
        for p in range(EB):
            nkp[0:64, p] = nk[2 * p]
            nkp[64:128, p] = nk[2 * p + 1]
        d["nkp"] = nkp
        d["nvf"] = np.ascontiguousarray(nv.reshape(1, DI))
        nbias = np.full((128, 1), NEG, np.float32)
        nbias[0, 0] = 0.0
        d["nbias"] = nbias
        shared.append(d)
    return shared


def host_in_maps(x, Wq, Wkv, Wout, null_kv, rot_q, rot_k):
    shared = host_prep_shared(Wq, Wkv, Wout, null_kv, rot_q, rot_k)
    in_maps = []
    for c in range(8):
        b, g = c // 2, c % 2
        m = dict(shared[g])
        m["xT"] = np.ascontiguousarray(
            np.asarray(x[b, g], np.float32).T).astype(ml_dtypes.bfloat16)
        in_maps.append(m)
    return in_maps


_NC_CACHE = {}


def _get_nc():
    if "nc" not in _NC_CACHE:
        nc = build_nc()
        nc.finalize()
        _NC_CACHE["nc"] = nc
    return _NC_CACHE["nc"]


def kernel(x, Wq, Wkv, Wout, null_kv, rot_q, rot_k):
    """Full-input entry point: shards over 8 NeuronCores, returns [B,G,N,D]."""
    in_maps = host_in_maps(np.asarray(x), np.asarray(Wq), np.asarray(Wkv),
                           np.asarray(Wout), np.asarray(null_kv),
                           np.asarray(rot_q), np.asarray(rot_k))
    nc = _get_nc()
    res = run_bass_kernel_spmd(nc, in_maps, core_ids=list(range(8)))
    out = np.empty((B, G, N, D), np.float32)
    for c in range(8):
        b, g = c // 2, c % 2
        out[b, g] = np.asarray(res.results[c]["out"]).T
    return out

